# revision 31
# baseline (speedup 1.0000x reference)
"""GPT-2 style attention block (B=2, S=2048, D=1024, H=16) on 8 TRN2 NeuronCores.

Sharding: tensor-parallel over heads + data-parallel over batch.
Cores 0-3 handle batch 0, cores 4-7 handle batch 1; each core owns 4 of the
16 heads (its 256-column slice of the qkv projection and the matching
256-row slice of c_proj_w). Each core produces a partial output
[S, D] (stored fp16) = ctx_heads @ c_proj_rows; the 4 partials per batch
are summed on the host (along with the bias rows, which are exactly zero
for the reference setup_inputs).

Design (fp16 dataflow; evolved from NTFF traces: serial baseline 295us ->
205us -> this interleaved version ~184us):
  * hs ships from the host already transposed to [D, S] fp16 (pure layout
    prep; on-device it used to cost 128 PE transposes + 16 DVE copies +
    17 ACT casts + 2x the DMA bytes). wqk ships with columns reordered
    [Q-hp0|K-hp0|Q-hp1|K-hp1] so the first needed half is one contiguous
    DMA. DMA priority: wqk first half, bqk, hsT s0-511, wv, hsT s512-1023,
    wqk second half, hsT s1024-2047, wp.
  * The PE starts on dummy transposes during the DMA wait: the HAM clock
    gate otherwise holds the first real matmuls at 1.2 GHz (K=4/8) and
    they gate everything downstream.
  * QK projection for (Q hp0, K hp0) x s0-511 runs first -> first scores
    at ~15us. All remaining projection work (vproj per row tile, qkproj
    per 128-col x 512-s block, early outproj blocks) is split into ~1-4us
    units drained into the attention stream.
  * One [128,1024] 2-buf PSUM ring ("big") carries scores AND every
    drained unit; units drain in PAIRS so scp tiles keep alternating
    buffers (an odd interleave count pins scores to one buffer and
    serializes score(k+1) behind exp(k)). cx pool 2x[65,2,512] holds the
    AV accumulators (ones-column of V_aug = softmax denominator in row
    64). 4+4 = 8 PSUM banks.
  * Attention block order qb0, qb1 (kt<=7 data only), qb3, qb2, each with
    AV emitted one iteration behind scores+exp so AV's wait on exp never
    blocks the next score pair. Exact causal trimming; one [128,128]
    upper-triangular fp16 mask handles diagonal tiles (DVE mul).
  * Normalize per (qb, hp): fp32 denominator copy, one fp32
    reciprocal_approx_fast (the 8-pass HW divide cost 3.3us per [1,512]),
    fp16 cast, 2 gpsimd partition_broadcasts, 2 DVE muls -> ctxT.
  * Outproj (4 matmuls per 128-row tile, wp stationary halves) drains one
    block late: qb0+qb1 under qb3, qb3 under qb2, qb2 at the tail; DVE
    does the PSUM->fp16 copies, then per-tile DMA out.
"""

import numpy as np

B, S, D, H = 2, 2048, 1024, 16
HD = D // H  # 64
N_CORES = 8
HPC = 4  # heads per core
GROUPS = 4  # cores per batch
HSL = HPC * HD  # 256: per-core head-column width

NDT = D // 128  # 8 contraction tiles
NRT = S // 128  # 16 row tiles
NQB = S // 512  # 4 query blocks
# wqk ships host-reordered as column blocks [ct0|ct2|ct1|ct3] so the two
# halves needed first (hp0's Q and K) are one contiguous DMA each
CT2COL = {0: 0, 2: 128, 1: 256, 3: 384}
CT2IDX = {0: 0, 2: 1, 1: 2, 3: 3}

_nc_cache = {}


def _build():
    from collections import deque

    import concourse.bacc as bacc
    import concourse.mybir as mybir
    import concourse.tile as tile
    from concourse.masks import make_upper_triangular

    f32 = mybir.dt.float32
    f16 = mybir.dt.float16

    nc = bacc.Bacc("TRN2", debug=False, num_devices=N_CORES)

    hsT_d = nc.dram_tensor("hsT", [D, S], f16, kind="ExternalInput")
    wqk = nc.dram_tensor("wqk", [D, 2 * HSL], f16, kind="ExternalInput")
    wv = nc.dram_tensor("wv", [D, HSL], f16, kind="ExternalInput")
    wp = nc.dram_tensor("wp", [HSL, D], f16, kind="ExternalInput")
    bqk = nc.dram_tensor("bqk", [2 * HSL], f32, kind="ExternalInput")
    outp = nc.dram_tensor("outp", [S, D], f16, kind="ExternalOutput")

    with tile.TileContext(nc) as tc:
        with (
            tc.tile_pool(name="persist", bufs=1) as persist,
            tc.tile_pool(name="es", bufs=10) as es_pool,
            tc.tile_pool(name="rb", bufs=4) as rb_pool,
            tc.tile_pool(name="ob", bufs=6) as ob_pool,
        ):
            # ---- persistent SBUF ----
            hsT = persist.tile([128, NDT, S], f16)  # [d%128, d//128, s]
            qkT = persist.tile([128, 4, S], f16)  # [Q hp0|Q hp1|K hp0|K hp1]
            vv = persist.tile([128, NRT, HPC * (HD + 1) + 76], f16)  # V aug (padded for 128-col stationaries)
            wqk_sb = persist.tile([128, NDT, 2 * HSL], f16)
            wv_sb = persist.tile([128, NDT, HSL], f16)
            wp_sb = persist.tile([128, 2, D], f16)
            bqk_sb = persist.tile([128, 4], f32)
            dmask = persist.tile([128, 128], f16)  # 1 where q(col) >= k(row)
            ctxT = persist.tile([128, 2, S], f16)

            make_upper_triangular(nc, dmask, val=1.0, diag=True)
            # ones columns of V_aug (data cols overwritten by vproj)
            nc.gpsimd.memset(vv, 1.0)

            hsT_src = hsT_d.rearrange("(t p) s -> p t s", p=128)
            wqk_src = wqk.rearrange("(t p) n -> p t n", p=128)

            # ---------- emitters ----------
            def emit_vproj(rt, pool, tag):
                pv = pool.tile([128, HSL], f32, tag=tag, name=f"pv{rt}")
                for dt in range(NDT):
                    nc.tensor.matmul(
                        pv,
                        hsT[:, dt, rt * 128 : (rt + 1) * 128],
                        wv_sb[:, dt, :],
                        start=(dt == 0),
                        stop=(dt == NDT - 1),
                    )
                vtgt = vv[:, rt, 0 : HPC * (HD + 1)].rearrange(
                    "p (h c) -> p h c", c=HD + 1
                )
                nc.vector.tensor_copy(
                    vtgt[:, :, 0:HD],
                    pv.rearrange("p (h c) -> p h c", c=HD),
                )

            def emit_qkproj(ct, sb, pool, tag):
                # one [128, 512] column-block of qkT for s-block sb
                pj = pool.tile(
                    [128, 512], f32, tag=tag, name=f"pj{ct}_{sb}"
                )
                co = CT2COL[ct]
                for dt in range(NDT):
                    nc.tensor.matmul(
                        pj,
                        wqk_sb[:, dt, co : co + 128],
                        hsT[:, dt, sb * 512 : (sb + 1) * 512],
                        start=(dt == 0),
                        stop=(dt == NDT - 1),
                    )
                nc.vector.tensor_scalar_add(
                    qkT[:, ct, sb * 512 : (sb + 1) * 512],
                    pj,
                    bqk_sb[:, CT2IDX[ct] : CT2IDX[ct] + 1],
                )

            def emit_scores_exp(qb, hp, kt, pool, tag):
                j = kt - 4 * qb
                w = 512 if j < 0 else 512 - 128 * j
                qo = 512 - w
                scp = pool.tile(
                    [128, 1024], f32, tag=tag, name=f"scp{qb}_{hp}_{kt}"
                )
                for hh in range(2):
                    nc.tensor.matmul(
                        scp[:, 512 * hh + qo : 512 * (hh + 1)],
                        qkT[
                            hh * 64 : (hh + 1) * 64,
                            2 + hp,
                            kt * 128 : (kt + 1) * 128,
                        ],
                        qkT[
                            hh * 64 : (hh + 1) * 64,
                            hp,
                            qb * 512 + qo : (qb + 1) * 512,
                        ],
                        start=True,
                        stop=True,
                        tile_position=(hh * 64, 0),
                    )
                es = es_pool.tile([128, 1024], f16, tag="es", name="es")
                scp3 = scp.rearrange("p (h c) -> p h c", c=512)
                es3 = es.rearrange("p (h c) -> p h c", c=512)
                nc.scalar.activation(
                    es3[:, :, qo:512],
                    scp3[:, :, qo:512],
                    mybir.ActivationFunctionType.Exp,
                    scale=float(1.0 / np.sqrt(HD)),
                )
                if j >= 0:
                    for hh in range(2):
                        nc.vector.tensor_mul(
                            es[:, 512 * hh + qo : 512 * hh + qo + 128],
                            es[:, 512 * hh + qo : 512 * hh + qo + 128],
                            dmask,
                        )
                return es

            def emit_av(qb, hp, kt, cxf, es):
                j = kt - 4 * qb
                w = 512 if j < 0 else 512 - 128 * j
                qo = 512 - w
                kmax = 4 * (qb + 1)
                for hh in range(2):
                    h = 2 * hp + hh
                    nc.tensor.matmul(
                        cxf[:, hh, qo:512],
                        vv[:, kt, h * (HD + 1) : h * (HD + 1) + 128],
                        es[:, 512 * hh + qo : 512 * (hh + 1)],
                        start=(kt == 0),
                        stop=(kt == kmax - 1),
                    )

            def emit_normalize(qb, hp, cx):
                # row 64 of cx holds the softmax denominator
                denf = rb_pool.tile([1, 1024], f32, tag="denf", name="denf")
                denf3 = denf.rearrange("p (h c) -> p h c", c=512)
                nc.scalar.copy(denf3, cx[64:65, :, :])
                recf = rb_pool.tile([1, 1024], f32, tag="recf", name="recf")
                nc.vector.reciprocal_approx_fast(recf, denf)
                rec = rb_pool.tile([1, 1024], f16, tag="rec", name="rec")
                rec3 = rec.rearrange("p (h c) -> p h c", c=512)
                nc.vector.tensor_copy(rec, recf)
                rbt = rb_pool.tile([64, 1024], f16, tag="rbt", name="rbt")
                rbt3 = rbt.rearrange("p (h c) -> p h c", c=512)
                for hh in range(2):
                    nc.gpsimd.partition_broadcast(
                        rbt3[:, hh, :], rec3[:, hh, :]
                    )
                for hh in range(2):
                    nc.vector.tensor_mul(
                        ctxT[
                            hh * 64 : hh * 64 + 64,
                            hp,
                            qb * 512 : (qb + 1) * 512,
                        ],
                        cx[0:64, hh, :],
                        rbt3[:, hh, :],
                    )

            def emit_outproj_half(mt, half, pool, tag):
                po = pool.tile(
                    [128, 512], f32, tag=tag, name=f"poh{mt}_{half}"
                )
                for ht in range(2):
                    nc.tensor.matmul(
                        po,
                        ctxT[:, ht, mt * 128 : (mt + 1) * 128],
                        wp_sb[:, ht, half * 512 : (half + 1) * 512],
                        start=(ht == 0),
                        stop=(ht == 1),
                    )
                ob = ob_pool.tile([128, 512], f16, tag="ob", name="ob")
                nc.vector.tensor_copy(ob, po)
                nc.sync.dma_start(
                    out=outp[
                        mt * 128 : (mt + 1) * 128,
                        half * 512 : (half + 1) * 512,
                    ],
                    in_=ob,
                )

            def emit_outproj_mt(mt, pool, tag):
                po = pool.tile([128, 1024], f32, tag=tag, name=f"po{mt}")
                for ht in range(2):
                    for half in range(2):
                        nc.tensor.matmul(
                            po[:, half * 512 : (half + 1) * 512],
                            ctxT[:, ht, mt * 128 : (mt + 1) * 128],
                            wp_sb[:, ht, half * 512 : (half + 1) * 512],
                            start=(ht == 0),
                            stop=(ht == 1),
                        )
                ob = ob_pool.tile([128, 1024], f16, tag="ob", name="ob")
                nc.vector.tensor_copy(ob, po)
                nc.sync.dma_start(
                    out=outp[mt * 128 : (mt + 1) * 128, :], in_=ob
                )

            # ---------- DMA issue (single FIFO queue -> priority order) --
            # wqk first half (hp0's Q,K columns) + bqk + hsT s0-511 first:
            # the first QK projections and scores gate everything else
            nc.sync.dma_start(
                out=wqk_sb[:, :, 0:256], in_=wqk_src[:, :, 0:256]
            )
            nc.sync.dma_start(
                out=bqk_sb, in_=bqk.rearrange("(t p) -> p t", p=128)
            )
            nc.sync.dma_start(
                out=hsT[:, :, 0:512], in_=hsT_src[:, :, 0:512]
            )
            nc.sync.dma_start(
                out=wv_sb, in_=wv.rearrange("(t p) n -> p t n", p=128)
            )
            nc.sync.dma_start(
                out=hsT[:, :, 512:1024], in_=hsT_src[:, :, 512:1024]
            )
            nc.sync.dma_start(
                out=wqk_sb[:, :, 256:512], in_=wqk_src[:, :, 256:512]
            )
            nc.sync.dma_start(
                out=hsT[:, :, 1024:2048], in_=hsT_src[:, :, 1024:2048]
            )
            nc.sync.dma_start(
                out=wp_sb, in_=wp.rearrange("(t p) n -> p t n", p=128)
            )

            # ---------- stage A: earliest QK projections ----------
            # One [128,1024] 2-buf PSUM ring ("big") carries scores AND
            # every interleaved projection/outproj unit for the whole
            # kernel; units are drained in PAIRS so scp tiles keep
            # alternating buffers (odd interleave counts would pin scores
            # to one buffer and serialize score(k+1) behind exp(k)).
            big_ctx = tc.tile_pool(name="big", bufs=2, space="PSUM")
            big = big_ctx.__enter__()
            cx_ctx = tc.tile_pool(name="cx", bufs=2, space="PSUM")
            cx_pool = cx_ctx.__enter__()

            warm_ctr = [0]

            def emit_warm(reps):
                # no-dependency PE filler: runs during stalls at phase
                # seams, keeping the HAM clock gate warm. Two ring tiles
                # preserve scp buffer parity.
                for wi in range(2):
                    warm_ctr[0] += 1
                    warm = big.tile(
                        [128, 128], f16, tag="big",
                        name=f"warm{warm_ctr[0]}",
                    )
                    for _ in range(reps):
                        nc.tensor.transpose(warm, dmask, dmask)

            # HAM warmup: the clock gate starts at K=4/8 (1.2 GHz) and
            # only unthrottles after ~3.4us of sustained PE activity; the
            # first QK projections otherwise run at half clock while also
            # gating everything downstream. Burn the DMA wait on dummy
            # matmuls (dmask is ready ~8us). Two ring tiles keep the
            # allocation count even for scp buffer-parity.
            emit_warm(20)

            for ct in (0, 2):
                emit_qkproj(ct, 0, big, "big")

            work = deque()

            def q_vproj(rts):
                for rt in rts:
                    work.append(lambda rt=rt: emit_vproj(rt, big, "big"))

            def q_qkproj(units):
                for ct, sb in units:
                    work.append(
                        lambda ct=ct, sb=sb: emit_qkproj(ct, sb, big, "big")
                    )

            def q_outproj(mts):
                for mt in mts:
                    work.append(
                        lambda mt=mt: emit_outproj_mt(mt, big, "big")
                    )

            it_counter = [0]

            def attention_block(qb, hp, drain_iters=None):
                cxf = cx_pool.tile(
                    [128, 2, 512], f32, tag="cx", name=f"cx{qb}_{hp}"
                )
                cx = cxf[0:65]
                es_prev = None
                kmax = 4 * (qb + 1)
                for kt in range(kmax):
                    es = emit_scores_exp(qb, hp, kt, big, "big")
                    if es_prev is not None:
                        emit_av(qb, hp, kt - 1, cxf, es_prev)
                    do_drain = (
                        drain_iters is None or it_counter[0] in drain_iters
                    )
                    if do_drain and len(work) >= 2:
                        work.popleft()()
                        work.popleft()()
                    it_counter[0] += 1
                    es_prev = es
                emit_av(qb, hp, kmax - 1, cxf, es_prev)
                emit_normalize(qb, hp, cx)

            # stage B fillers (pairs; drained 1 pair/iter)
            q_vproj((0, 1, 2, 3))
            q_qkproj([(1, 0), (3, 0)])  # qb0-hp1 Q/K
            q_qkproj([(0, 1), (2, 1)])  # qb1-hp0 Q/K
            q_vproj((4, 5, 6, 7))
            q_qkproj([(1, 1), (3, 1)])  # qb1-hp1
            q_qkproj([(0, 3), (2, 3)])  # qb3-hp0 Q / K kt12-15
            q_qkproj([(1, 3), (2, 2)])  # qb3-hp1 Q; K-hp0 kt8-11

            b_drains = {0, 1, 3, 6, 9, 11, 13, 15, 17}
            attention_block(0, 0, b_drains)
            attention_block(0, 1, b_drains)
            attention_block(1, 0, b_drains)
            attention_block(1, 1, b_drains)

            # stage C deferred work (all pair-aligned)
            q_qkproj([(0, 2), (1, 2)])  # qb2 Q
            q_vproj((8, 9, 10, 11, 12, 13, 14, 15))
            q_qkproj([(3, 2), (3, 3)])  # K-hp1 kt8-15
            q_outproj(range(0, 8))  # qb0+qb1 outproj

            it_counter[0] = 0
            c_drains = set(range(1, 64, 2))
            attention_block(3, 0, c_drains)
            attention_block(3, 1, c_drains)
            emit_warm(5)
            q_outproj(range(12, 16))  # qb3 outproj
            attention_block(2, 0, c_drains)
            attention_block(2, 1, c_drains)
            emit_warm(8)
            q_outproj(range(8, 12))  # qb2 outproj
            while work:
                work.popleft()()

            cx_ctx.__exit__(None, None, None)
            big_ctx.__exit__(None, None, None)

    nc.compile()
    return nc


def build_kernel(matmul_dtype=None, av_dtype=None):
    # single fp16 variant; dtype args accepted for harness compat
    if "k" not in _nc_cache:
        _nc_cache["k"] = _build()
    return _nc_cache["k"]


def make_in_maps(
    hidden_states, c_attn_w, c_attn_b, c_proj_w, c_proj_b,
    matmul_dtype=None, av_dtype=None,
):
    hidden_states = np.asarray(hidden_states, dtype=np.float32)
    c_attn_w = np.asarray(c_attn_w, dtype=np.float32)
    c_attn_b = np.asarray(c_attn_b, dtype=np.float32)
    c_proj_w = np.asarray(c_proj_w, dtype=np.float32)
    c_proj_b = np.asarray(c_proj_b, dtype=np.float32)

    in_maps = []
    for c in range(N_CORES):
        b, g = divmod(c, GROUPS)
        cs = slice(g * HSL, (g + 1) * HSL)
        wq = c_attn_w[:, g * HSL : (g + 1) * HSL]
        wk = c_attn_w[:, D + g * HSL : D + (g + 1) * HSL]
        wvs = c_attn_w[:, 2 * D + g * HSL : 2 * D + (g + 1) * HSL]
        bq = c_attn_b[g * HSL : (g + 1) * HSL]
        bk = c_attn_b[D + g * HSL : D + (g + 1) * HSL]
        bv = c_attn_b[2 * D + g * HSL : 2 * D + (g + 1) * HSL]
        wps = c_proj_w[cs, :]
        rr = bv.astype(np.float64) @ wps.astype(np.float64)
        if g == 0:
            rr = rr + c_proj_b
        in_maps.append(
            {
                "hsT": np.ascontiguousarray(
                    hidden_states[b].T.astype(np.float16)
                ),
                "wqk": np.ascontiguousarray(
                    np.concatenate(
                        [wq[:, :128], wk[:, :128], wq[:, 128:], wk[:, 128:]],
                        axis=1,
                    ).astype(np.float16)
                ),
                "wv": np.ascontiguousarray(wvs.astype(np.float16)),
                "wp": np.ascontiguousarray(wps.astype(np.float16)),
                "bqk": np.ascontiguousarray(
                    np.concatenate([bq[:128], bk[:128], bq[128:], bk[128:]])
                ),
                "_rrow": np.ascontiguousarray(rr.astype(np.float32)),
            }
        )
    return in_maps


def kernel(
    hidden_states,
    c_attn_w,
    c_attn_b,
    c_proj_w,
    c_proj_b,
    causal_mask=None,
    **_unused,
):
    from concourse.bass_utils import run_bass_kernel_spmd

    nc = build_kernel()
    in_maps = make_in_maps(
        hidden_states, c_attn_w, c_attn_b, c_proj_w, c_proj_b
    )
    rrows = [m.pop("_rrow") for m in in_maps]
    res = run_bass_kernel_spmd(nc, in_maps, list(range(N_CORES)))
    out = np.zeros((B, S, D), dtype=np.float32)
    for c in range(N_CORES):
        out[c // GROUPS] += res.results[c]["outp"].astype(np.float32)
        out[c // GROUPS] += rrows[c]
    return out


# revision 32
# speedup vs baseline: 1.0246x; 1.0246x over previous
"""GPT-2 style attention block (B=2, S=2048, D=1024, H=16) on 8 TRN2 NeuronCores.

Sharding: tensor-parallel over heads + data-parallel over batch.
Cores 0-3 handle batch 0, cores 4-7 handle batch 1; each core owns 4 of the
16 heads (its 256-column slice of the qkv projection and the matching
256-row slice of c_proj_w). Each core produces a partial output
[S, D] (stored fp16) = ctx_heads @ c_proj_rows; the 4 partials per batch
are summed on the host (along with the bias rows, which are exactly zero
for the reference setup_inputs).

Design (fp16 dataflow; evolved from NTFF traces: serial baseline 295us ->
205us -> this interleaved version ~184us):
  * hs ships from the host already transposed to [D, S] fp16 (pure layout
    prep; on-device it used to cost 128 PE transposes + 16 DVE copies +
    17 ACT casts + 2x the DMA bytes). wqk ships with columns reordered
    [Q-hp0|K-hp0|Q-hp1|K-hp1] so the first needed half is one contiguous
    DMA. DMA priority: wqk first half, bqk, hsT s0-511, wv, hsT s512-1023,
    wqk second half, hsT s1024-2047, wp.
  * The PE starts on dummy transposes during the DMA wait: the HAM clock
    gate otherwise holds the first real matmuls at 1.2 GHz (K=4/8) and
    they gate everything downstream.
  * QK projection for (Q hp0, K hp0) x s0-511 runs first -> first scores
    at ~15us. All remaining projection work (vproj per row tile, qkproj
    per 128-col x 512-s block, early outproj blocks) is split into ~1-4us
    units drained into the attention stream.
  * One [128,1024] 2-buf PSUM ring ("big") carries scores AND every
    drained unit; units drain in PAIRS so scp tiles keep alternating
    buffers (an odd interleave count pins scores to one buffer and
    serializes score(k+1) behind exp(k)). cx pool 2x[65,2,512] holds the
    AV accumulators (ones-column of V_aug = softmax denominator in row
    64). 4+4 = 8 PSUM banks.
  * Attention block order qb0, qb1 (kt<=7 data only), qb3, qb2, each with
    AV emitted one iteration behind scores+exp so AV's wait on exp never
    blocks the next score pair. Exact causal trimming; one [128,128]
    upper-triangular fp16 mask handles diagonal tiles (DVE mul).
  * Normalize per (qb, hp): fp32 denominator copy, one fp32
    reciprocal_approx_fast (the 8-pass HW divide cost 3.3us per [1,512]),
    fp16 cast, 2 gpsimd partition_broadcasts, 2 DVE muls -> ctxT.
  * Outproj (4 matmuls per 128-row tile, wp stationary halves) drains one
    block late: qb0+qb1 under qb3, qb3 under qb2, qb2 at the tail; DVE
    does the PSUM->fp16 copies, then per-tile DMA out.
"""

import numpy as np

B, S, D, H = 2, 2048, 1024, 16
HD = D // H  # 64
N_CORES = 8
HPC = 4  # heads per core
GROUPS = 4  # cores per batch
HSL = HPC * HD  # 256: per-core head-column width

NDT = D // 128  # 8 contraction tiles
NRT = S // 128  # 16 row tiles
NQB = S // 512  # 4 query blocks
# wqk ships host-reordered as column blocks [ct0|ct2|ct1|ct3] so the two
# halves needed first (hp0's Q and K) are one contiguous DMA each
CT2COL = {0: 0, 2: 128, 1: 256, 3: 384}
CT2IDX = {0: 0, 2: 1, 1: 2, 3: 3}

_nc_cache = {}


def _build():
    from collections import deque

    import concourse.bacc as bacc
    import concourse.mybir as mybir
    import concourse.tile as tile
    from concourse.masks import make_upper_triangular

    f32 = mybir.dt.float32
    f16 = mybir.dt.float16

    nc = bacc.Bacc("TRN2", debug=False, num_devices=N_CORES)

    hsT_d = nc.dram_tensor("hsT", [D, S], f16, kind="ExternalInput")
    wqk = nc.dram_tensor("wqk", [D, 2 * HSL], f16, kind="ExternalInput")
    wv = nc.dram_tensor("wv", [D, HSL], f16, kind="ExternalInput")
    wp = nc.dram_tensor("wp", [HSL, D], f16, kind="ExternalInput")
    bqk = nc.dram_tensor("bqk", [2 * HSL], f32, kind="ExternalInput")
    outp = nc.dram_tensor("outp", [S, D], f16, kind="ExternalOutput")

    with tile.TileContext(nc) as tc:
        with (
            tc.tile_pool(name="persist", bufs=1) as persist,
            tc.tile_pool(name="es", bufs=10) as es_pool,
            tc.tile_pool(name="rb", bufs=4) as rb_pool,
            tc.tile_pool(name="ob", bufs=6) as ob_pool,
        ):
            # ---- persistent SBUF ----
            hsT = persist.tile([128, NDT, S], f16)  # [d%128, d//128, s]
            qkT = persist.tile([128, 4, S], f16)  # [Q hp0|Q hp1|K hp0|K hp1]
            vv = persist.tile([128, NRT, HPC * (HD + 1) + 76], f16)  # V aug (padded for 128-col stationaries)
            wqk_sb = persist.tile([128, NDT, 2 * HSL], f16)
            wv_sb = persist.tile([128, NDT, HSL], f16)
            wp_sb = persist.tile([128, 2, D], f16)
            bqk_sb = persist.tile([128, 4], f32)
            dmask = persist.tile([128, 128], f16)  # 1 where q(col) >= k(row)
            ctxT = persist.tile([128, 2, S], f16)

            make_upper_triangular(nc, dmask, val=1.0, diag=True)
            # ones columns of V_aug (data cols overwritten by vproj)
            nc.gpsimd.memset(vv, 1.0)

            hsT_src = hsT_d.rearrange("(t p) s -> p t s", p=128)
            wqk_src = wqk.rearrange("(t p) n -> p t n", p=128)

            # ---------- emitters ----------
            def emit_vproj(rt, pool, tag):
                pv = pool.tile([128, HSL], f32, tag=tag, name=f"pv{rt}")
                for dt in range(NDT):
                    nc.tensor.matmul(
                        pv,
                        hsT[:, dt, rt * 128 : (rt + 1) * 128],
                        wv_sb[:, dt, :],
                        start=(dt == 0),
                        stop=(dt == NDT - 1),
                    )
                vtgt = vv[:, rt, 0 : HPC * (HD + 1)].rearrange(
                    "p (h c) -> p h c", c=HD + 1
                )
                nc.vector.tensor_copy(
                    vtgt[:, :, 0:HD],
                    pv.rearrange("p (h c) -> p h c", c=HD),
                )

            def emit_qkproj(ct, sb, pool, tag):
                # one [128, 512] column-block of qkT for s-block sb
                pj = pool.tile(
                    [128, 512], f32, tag=tag, name=f"pj{ct}_{sb}"
                )
                co = CT2COL[ct]
                for dt in range(NDT):
                    nc.tensor.matmul(
                        pj,
                        wqk_sb[:, dt, co : co + 128],
                        hsT[:, dt, sb * 512 : (sb + 1) * 512],
                        start=(dt == 0),
                        stop=(dt == NDT - 1),
                    )
                nc.vector.tensor_scalar_add(
                    qkT[:, ct, sb * 512 : (sb + 1) * 512],
                    pj,
                    bqk_sb[:, CT2IDX[ct] : CT2IDX[ct] + 1],
                )

            def emit_scores_exp(qb, hp, kt, pool, tag):
                j = kt - 4 * qb
                w = 512 if j < 0 else 512 - 128 * j
                qo = 512 - w
                scp = pool.tile(
                    [128, 1024], f32, tag=tag, name=f"scp{qb}_{hp}_{kt}"
                )
                for hh in range(2):
                    nc.tensor.matmul(
                        scp[:, 512 * hh + qo : 512 * (hh + 1)],
                        qkT[
                            hh * 64 : (hh + 1) * 64,
                            2 + hp,
                            kt * 128 : (kt + 1) * 128,
                        ],
                        qkT[
                            hh * 64 : (hh + 1) * 64,
                            hp,
                            qb * 512 + qo : (qb + 1) * 512,
                        ],
                        start=True,
                        stop=True,
                        tile_position=(hh * 64, 0),
                    )
                es = es_pool.tile([128, 1024], f16, tag="es", name="es")
                scp3 = scp.rearrange("p (h c) -> p h c", c=512)
                es3 = es.rearrange("p (h c) -> p h c", c=512)
                nc.scalar.activation(
                    es3[:, :, qo:512],
                    scp3[:, :, qo:512],
                    mybir.ActivationFunctionType.Exp,
                    scale=float(1.0 / np.sqrt(HD)),
                )
                if j >= 0:
                    for hh in range(2):
                        nc.vector.tensor_mul(
                            es[:, 512 * hh + qo : 512 * hh + qo + 128],
                            es[:, 512 * hh + qo : 512 * hh + qo + 128],
                            dmask,
                        )
                return es

            def emit_av(qb, hp, kt, cxf, es):
                j = kt - 4 * qb
                w = 512 if j < 0 else 512 - 128 * j
                qo = 512 - w
                kmax = 4 * (qb + 1)
                for hh in range(2):
                    h = 2 * hp + hh
                    nc.tensor.matmul(
                        cxf[:, hh, qo:512],
                        vv[:, kt, h * (HD + 1) : h * (HD + 1) + 128],
                        es[:, 512 * hh + qo : 512 * (hh + 1)],
                        start=(kt == 0),
                        stop=(kt == kmax - 1),
                    )

            def emit_normalize(qb, hp, cx):
                # row 64 of cx holds the softmax denominator
                denf = rb_pool.tile([1, 1024], f32, tag="denf", name="denf")
                denf3 = denf.rearrange("p (h c) -> p h c", c=512)
                nc.vector.tensor_copy(denf3, cx[64:65, :, :])
                recf = rb_pool.tile([1, 1024], f32, tag="recf", name="recf")
                nc.vector.reciprocal_approx_fast(recf, denf)
                rec = rb_pool.tile([1, 1024], f16, tag="rec", name="rec")
                rec3 = rec.rearrange("p (h c) -> p h c", c=512)
                nc.vector.tensor_copy(rec, recf)
                rbt = rb_pool.tile([64, 1024], f16, tag="rbt", name="rbt")
                rbt3 = rbt.rearrange("p (h c) -> p h c", c=512)
                for hh in range(2):
                    nc.gpsimd.partition_broadcast(
                        rbt3[:, hh, :], rec3[:, hh, :]
                    )
                for hh in range(2):
                    nc.vector.tensor_mul(
                        ctxT[
                            hh * 64 : hh * 64 + 64,
                            hp,
                            qb * 512 : (qb + 1) * 512,
                        ],
                        cx[0:64, hh, :],
                        rbt3[:, hh, :],
                    )

            def emit_outproj_half(mt, half, pool, tag):
                po = pool.tile(
                    [128, 512], f32, tag=tag, name=f"poh{mt}_{half}"
                )
                for ht in range(2):
                    nc.tensor.matmul(
                        po,
                        ctxT[:, ht, mt * 128 : (mt + 1) * 128],
                        wp_sb[:, ht, half * 512 : (half + 1) * 512],
                        start=(ht == 0),
                        stop=(ht == 1),
                    )
                ob = ob_pool.tile([128, 512], f16, tag="ob", name="ob")
                nc.vector.tensor_copy(ob, po)
                nc.sync.dma_start(
                    out=outp[
                        mt * 128 : (mt + 1) * 128,
                        half * 512 : (half + 1) * 512,
                    ],
                    in_=ob,
                )

            def emit_outproj_mt(mt, pool, tag):
                po = pool.tile([128, 1024], f32, tag=tag, name=f"po{mt}")
                for ht in range(2):
                    for half in range(2):
                        nc.tensor.matmul(
                            po[:, half * 512 : (half + 1) * 512],
                            ctxT[:, ht, mt * 128 : (mt + 1) * 128],
                            wp_sb[:, ht, half * 512 : (half + 1) * 512],
                            start=(ht == 0),
                            stop=(ht == 1),
                        )
                ob = ob_pool.tile([128, 1024], f16, tag="ob", name="ob")
                nc.vector.tensor_copy(ob, po)
                nc.sync.dma_start(
                    out=outp[mt * 128 : (mt + 1) * 128, :], in_=ob
                )

            # ---------- DMA issue (single FIFO queue -> priority order) --
            # wqk first half (hp0's Q,K columns) + bqk + hsT s0-511 first:
            # the first QK projections and scores gate everything else
            nc.sync.dma_start(
                out=wqk_sb[:, :, 0:256], in_=wqk_src[:, :, 0:256]
            )
            nc.sync.dma_start(
                out=bqk_sb, in_=bqk.rearrange("(t p) -> p t", p=128)
            )
            nc.sync.dma_start(
                out=hsT[:, :, 0:512], in_=hsT_src[:, :, 0:512]
            )
            nc.sync.dma_start(
                out=wv_sb, in_=wv.rearrange("(t p) n -> p t n", p=128)
            )
            nc.sync.dma_start(
                out=hsT[:, :, 512:1024], in_=hsT_src[:, :, 512:1024]
            )
            nc.sync.dma_start(
                out=wqk_sb[:, :, 256:512], in_=wqk_src[:, :, 256:512]
            )
            nc.sync.dma_start(
                out=hsT[:, :, 1024:2048], in_=hsT_src[:, :, 1024:2048]
            )
            nc.sync.dma_start(
                out=wp_sb, in_=wp.rearrange("(t p) n -> p t n", p=128)
            )

            # ---------- stage A: earliest QK projections ----------
            # One [128,1024] 2-buf PSUM ring ("big") carries scores AND
            # every interleaved projection/outproj unit for the whole
            # kernel; units are drained in PAIRS so scp tiles keep
            # alternating buffers (odd interleave counts would pin scores
            # to one buffer and serialize score(k+1) behind exp(k)).
            big_ctx = tc.tile_pool(name="big", bufs=2, space="PSUM")
            big = big_ctx.__enter__()
            cx_ctx = tc.tile_pool(name="cx", bufs=2, space="PSUM")
            cx_pool = cx_ctx.__enter__()

            warm_ctr = [0]

            def emit_warm(reps):
                # no-dependency PE filler: runs during stalls at phase
                # seams, keeping the HAM clock gate warm. Two ring tiles
                # preserve scp buffer parity.
                for wi in range(2):
                    warm_ctr[0] += 1
                    warm = big.tile(
                        [128, 128], f16, tag="big",
                        name=f"warm{warm_ctr[0]}",
                    )
                    for _ in range(reps):
                        nc.tensor.transpose(warm, dmask, dmask)

            # HAM warmup: the clock gate starts at K=4/8 (1.2 GHz) and
            # only unthrottles after ~3.4us of sustained PE activity; the
            # first QK projections otherwise run at half clock while also
            # gating everything downstream. Burn the DMA wait on dummy
            # matmuls (dmask is ready ~8us). Two ring tiles keep the
            # allocation count even for scp buffer-parity.
            emit_warm(20)

            for ct in (0, 2):
                emit_qkproj(ct, 0, big, "big")

            work = deque()

            def q_vproj(rts):
                for rt in rts:
                    work.append(lambda rt=rt: emit_vproj(rt, big, "big"))

            def q_qkproj(units):
                for ct, sb in units:
                    work.append(
                        lambda ct=ct, sb=sb: emit_qkproj(ct, sb, big, "big")
                    )

            def q_outproj(mts):
                for mt in mts:
                    work.append(
                        lambda mt=mt: emit_outproj_mt(mt, big, "big")
                    )

            it_counter = [0]

            def attention_block(qb, hp, drain_iters=None):
                cxf = cx_pool.tile(
                    [128, 2, 512], f32, tag="cx", name=f"cx{qb}_{hp}"
                )
                cx = cxf[0:65]
                es_prev = None
                kmax = 4 * (qb + 1)
                for kt in range(kmax):
                    es = emit_scores_exp(qb, hp, kt, big, "big")
                    if es_prev is not None:
                        emit_av(qb, hp, kt - 1, cxf, es_prev)
                    do_drain = (
                        drain_iters is None or it_counter[0] in drain_iters
                    )
                    if do_drain and len(work) >= 2:
                        work.popleft()()
                        work.popleft()()
                    it_counter[0] += 1
                    es_prev = es
                emit_av(qb, hp, kmax - 1, cxf, es_prev)
                emit_normalize(qb, hp, cx)

            # stage B fillers (pairs; drained 1 pair/iter)
            q_vproj((0, 1, 2, 3))
            q_qkproj([(1, 0), (3, 0)])  # qb0-hp1 Q/K
            q_qkproj([(0, 1), (2, 1)])  # qb1-hp0 Q/K
            q_vproj((4, 5, 6, 7))
            q_qkproj([(1, 1), (3, 1)])  # qb1-hp1
            q_qkproj([(0, 3), (2, 3)])  # qb3-hp0 Q / K kt12-15
            q_qkproj([(1, 3), (2, 2)])  # qb3-hp1 Q; K-hp0 kt8-11

            b_drains = {0, 1, 3, 6, 9, 11, 13, 15, 17}
            attention_block(0, 0, b_drains)
            attention_block(0, 1, b_drains)
            attention_block(1, 0, b_drains)
            attention_block(1, 1, b_drains)

            # stage C deferred work (all pair-aligned)
            q_qkproj([(0, 2), (1, 2)])  # qb2 Q
            q_vproj((8, 9, 10, 11, 12, 13, 14, 15))
            q_qkproj([(3, 2), (3, 3)])  # K-hp1 kt8-15
            q_outproj(range(0, 8))  # qb0+qb1 outproj

            it_counter[0] = 0
            c_drains = set(range(1, 64, 2))
            attention_block(3, 0, c_drains)
            attention_block(3, 1, c_drains)
            emit_warm(5)
            q_outproj(range(12, 16))  # qb3 outproj
            attention_block(2, 0, c_drains)
            attention_block(2, 1, c_drains)
            emit_warm(8)
            q_outproj(range(8, 12))  # qb2 outproj
            while work:
                work.popleft()()

            cx_ctx.__exit__(None, None, None)
            big_ctx.__exit__(None, None, None)

    nc.compile()
    return nc


def build_kernel(matmul_dtype=None, av_dtype=None):
    # single fp16 variant; dtype args accepted for harness compat
    if "k" not in _nc_cache:
        _nc_cache["k"] = _build()
    return _nc_cache["k"]


def make_in_maps(
    hidden_states, c_attn_w, c_attn_b, c_proj_w, c_proj_b,
    matmul_dtype=None, av_dtype=None,
):
    hidden_states = np.asarray(hidden_states, dtype=np.float32)
    c_attn_w = np.asarray(c_attn_w, dtype=np.float32)
    c_attn_b = np.asarray(c_attn_b, dtype=np.float32)
    c_proj_w = np.asarray(c_proj_w, dtype=np.float32)
    c_proj_b = np.asarray(c_proj_b, dtype=np.float32)

    in_maps = []
    for c in range(N_CORES):
        b, g = divmod(c, GROUPS)
        cs = slice(g * HSL, (g + 1) * HSL)
        wq = c_attn_w[:, g * HSL : (g + 1) * HSL]
        wk = c_attn_w[:, D + g * HSL : D + (g + 1) * HSL]
        wvs = c_attn_w[:, 2 * D + g * HSL : 2 * D + (g + 1) * HSL]
        bq = c_attn_b[g * HSL : (g + 1) * HSL]
        bk = c_attn_b[D + g * HSL : D + (g + 1) * HSL]
        bv = c_attn_b[2 * D + g * HSL : 2 * D + (g + 1) * HSL]
        wps = c_proj_w[cs, :]
        rr = bv.astype(np.float64) @ wps.astype(np.float64)
        if g == 0:
            rr = rr + c_proj_b
        in_maps.append(
            {
                "hsT": np.ascontiguousarray(
                    hidden_states[b].T.astype(np.float16)
                ),
                "wqk": np.ascontiguousarray(
                    np.concatenate(
                        [wq[:, :128], wk[:, :128], wq[:, 128:], wk[:, 128:]],
                        axis=1,
                    ).astype(np.float16)
                ),
                "wv": np.ascontiguousarray(wvs.astype(np.float16)),
                "wp": np.ascontiguousarray(wps.astype(np.float16)),
                "bqk": np.ascontiguousarray(
                    np.concatenate([bq[:128], bk[:128], bq[128:], bk[128:]])
                ),
                "_rrow": np.ascontiguousarray(rr.astype(np.float32)),
            }
        )
    return in_maps


def kernel(
    hidden_states,
    c_attn_w,
    c_attn_b,
    c_proj_w,
    c_proj_b,
    causal_mask=None,
    **_unused,
):
    from concourse.bass_utils import run_bass_kernel_spmd

    nc = build_kernel()
    in_maps = make_in_maps(
        hidden_states, c_attn_w, c_attn_b, c_proj_w, c_proj_b
    )
    rrows = [m.pop("_rrow") for m in in_maps]
    res = run_bass_kernel_spmd(nc, in_maps, list(range(N_CORES)))
    out = np.zeros((B, S, D), dtype=np.float32)
    for c in range(N_CORES):
        out[c // GROUPS] += res.results[c]["outp"].astype(np.float32)
        out[c // GROUPS] += rrows[c]
    return out
